# revision 18
# baseline (speedup 1.0000x reference)
"""Trainium2 Bass kernel for nn_KeypointLoss.

Reference computation (S=nstack+1=5, B=16, C=11, H=W=128, NCLASS=11, F=7):
  heat_l[b, s]   = sum_c mean_{h,w} (heat_preds[s,b,c,h,w] - heatmaps[s,b,c,h,w])^2
  labels_l[b, s] = sum_{c,f<7} (label_preds[s,b,c,f] - labels[b,c,f])^2
Returns (heat_l [B,S], labels_l [B,S]).

Sharding: data parallel over batch — core i handles b in {2i, 2i+1}.
Per core: read 2 x 7.2 MB of heatmap data (memory-bound). Raw Bass,
hand-scheduled. The DMA stream runs at the per-core HBM ceiling
(~400 GB/s, all 16 SDMA engines saturated), so the optimization targets
are the head (time to first byte) and especially the tail (work after
the last byte lands):
  * heat loads double-buffered on the SP HWDGE queue; diff on VectorE,
    square + free-axis reduce on ScalarE (activation Square with
    accum_out, scale 1/H folds the mean over (h, w)).
  * chunk 8 keeps its full-size DMA (5632 B descriptors) but its
    compute is split into two 704-col tiles so the ScalarE chain is
    never clogged by a 1.7 us ACT when the last chunk's tiles arrive.
  * chunk 9 is DMA-split [704, 352, 352] (descriptors 2816/1408 B) so
    its compute overlaps the stream tail and the post-stream chain is
    one small TT + ACT.
  * the 128-way partition reduction runs on-core: PE matmul with the
    framework's const ones[128,1] (cols 0..11 as soon as they are
    ready, the last col right after its ACT), PSUM -> SBUF copy on
    ScalarE, then a single-descriptor 52 B store from the idle SP
    queue. Label losses ride the ACT HWDGE queue early and are hidden
    under the stream.
"""

import sys

if "/opt/trn_rl_repo" not in sys.path:
    sys.path.insert(0, "/opt/trn_rl_repo")

import numpy as np

import concourse.bass as bass
import concourse.mybir as mybir
from concourse.bass_utils import run_bass_kernel_spmd

S, B, C, H, W = 5, 16, 11, 128, 128
NCLASS, F = 11, 7
NCORES = 8
BL = B // NCORES          # batch samples per core (2)
P = 128                   # SBUF partitions
X = C * H * W // P        # 1408 free elements per partition per (s, b)
LF = NCLASS * F           # 77 label features per (s, b)
SB = S * BL               # 10 (s, b) pairs per core
NFULL = SB - 2            # chunks 0..7: one transfer pair, one TT, one ACT
C8SUBS = [704, 704]       # chunk 8 DMA split (cols)
SUBS = [704, 480, 224]    # chunk 9 DMA split (cols)
# compute tiles after the full chunks: one per sub-transfer pair
TILES = ([(8, c, w) for c, w in zip([0, 704], C8SUBS)] +
         [(9, c, w) for c, w in zip([0, 704, 1184], SUBS)])
NCOLS = NFULL + len(TILES)    # 13 accumulator columns

f32 = mybir.dt.float32
SQUARE = mybir.ActivationFunctionType.Square
SUB = mybir.AluOpType.subtract


def build_program():
    nc = bass.Bass("TRN2")
    hp = nc.dram_tensor("hp", [S, BL, C, H, W], f32, kind="ExternalInput")
    hm = nc.dram_tensor("hm", [S, BL, C, H, W], f32, kind="ExternalInput")
    lp = nc.dram_tensor("lp", [S, BL, NCLASS, F], f32, kind="ExternalInput")
    lb = nc.dram_tensor("lb", [BL, NCLASS, F], f32, kind="ExternalInput")
    # heat_out: reduced loss per accumulator column; host folds cols 8..9
    # into chunk 8 and cols 10..12 into chunk 9
    heat_out = nc.dram_tensor("heat_out", [NCOLS], f32, kind="ExternalOutput")
    label_out = nc.dram_tensor("label_out", [S, BL], f32, kind="ExternalOutput")

    # chunk k = (s, b) pair: 0.7 MB contiguous per tensor. The (c,h,w)
    # reduction covers the whole chunk, so a flat [(p x)] -> [p, x] layout
    # (contiguous 5632 B runs per partition) is valid and DMA-optimal.
    def chunk_src(t, k):
        return t[k // BL, k % BL].rearrange("c h w -> (c h w)").rearrange(
            "(p x) -> p x", p=P)

    NBUF = 3
    from contextlib import ExitStack
    with ExitStack() as es:
        a_t = es.enter_context(nc.sbuf_tensor([P, NBUF, X], f32))
        b_t = es.enter_context(nc.sbuf_tensor([P, NBUF, X], f32))
        d_t = es.enter_context(nc.sbuf_tensor([P, NBUF, X], f32))
        acc = es.enter_context(nc.sbuf_tensor([P, NCOLS], f32))
        red = es.enter_context(nc.sbuf_tensor([1, NCOLS], f32))
        lp_t = es.enter_context(nc.sbuf_tensor([SB, LF], f32))
        lb_t = es.enter_context(nc.sbuf_tensor([SB, LF], f32))
        lacc = es.enter_context(nc.sbuf_tensor([SB, 1], f32))
        pacc = es.enter_context(nc.psum_tensor("pacc", [1, NCOLS], f32))
        # one sem per heat transfer pair: DMA completions are unordered
        # across transfers, so cumulative counting on one sem is racy
        dma_sems = [es.enter_context(nc.semaphore(f"dma_sem{k}"))
                    for k in range(NFULL)]
        c8_sems = [es.enter_context(nc.semaphore(f"c8_sem{j}"))
                   for j in range(len(C8SUBS))]
        sub_sems = [es.enter_context(nc.semaphore(f"sub_sem{j}"))
                    for j in range(len(SUBS))]
        ldma_sem = es.enter_context(nc.semaphore("ldma_sem"))  # label loads
        odma_sem = es.enter_context(nc.semaphore("odma_sem"))  # output stores
        dve_sem = es.enter_context(nc.semaphore("dve_sem"))    # +1 per heat TT
        ldve_sem = es.enter_context(nc.semaphore("ldve_sem"))  # +1 label TT
        act_sem = es.enter_context(nc.semaphore("act_sem"))    # +1 per heat ACT
        lact_sem = es.enter_context(nc.semaphore("lact_sem"))  # +1 label ACT
        mm_sem = es.enter_context(nc.semaphore("mm_sem"))      # +1 per matmul
        cp_sem = es.enter_context(nc.semaphore("cp_sem"))      # PSUM->SBUF copy
        ones = nc.const_aps.aps[(f32, 1.0)]                    # [128, 1]
        block = es.enter_context(nc.Block())

        @block.sync
        def _(sync):
            # heat loads only — keep this queue dense
            for k in range(NFULL):            # chunks 0..7 whole
                if k >= NBUF:
                    # a/b slot free once the TT of chunk k-NBUF ran
                    sync.wait_ge(dve_sem, k - NBUF + 1)
                sync.dma_start(out=a_t[:, k % NBUF],
                               in_=chunk_src(hp, k)).then_inc(dma_sems[k], 16)
                sync.dma_start(out=b_t[:, k % NBUF],
                               in_=chunk_src(hm, k)).then_inc(dma_sems[k], 16)
            # chunk 8 in 2 column sub-transfers (slot 2)
            sync.wait_ge(dve_sem, 8 - NBUF + 1)
            c0 = 0
            for j, w in enumerate(C8SUBS):
                sync.dma_start(out=a_t[:, 2, c0:c0 + w],
                               in_=chunk_src(hp, 8)[:, c0:c0 + w]
                               ).then_inc(c8_sems[j], 16)
                sync.dma_start(out=b_t[:, 2, c0:c0 + w],
                               in_=chunk_src(hm, 8)[:, c0:c0 + w]
                               ).then_inc(c8_sems[j], 16)
                c0 += w
            # chunk 9 in 3 column sub-transfers (slot 0)
            sync.wait_ge(dve_sem, SB - NBUF)
            c0 = 0
            for j, w in enumerate(SUBS):
                sync.dma_start(out=a_t[:, 0, c0:c0 + w],
                               in_=chunk_src(hp, SB - 1)[:, c0:c0 + w]
                               ).then_inc(sub_sems[j], 16)
                sync.dma_start(out=b_t[:, 0, c0:c0 + w],
                               in_=chunk_src(hm, SB - 1)[:, c0:c0 + w]
                               ).then_inc(sub_sems[j], 16)
                c0 += w
            # final 52 B store of the reduced losses (single descriptor)
            sync.wait_ge(cp_sem, 1)
            sync.dma_start(out=heat_out[:],
                           in_=red[0:1, :]).then_inc(odma_sem, 16)
            sync.wait_ge(odma_sem, 32)

        @block.vector
        def _(vector):
            # heat TTs lead — the label path must never gate the heat
            # pipeline. The label TT slips in after chunk 2, when its
            # loads are long done and the DVE has slack under the DMA
            # cadence.
            for k in range(NFULL):
                vector.wait_ge(dma_sems[k], 32)
                if k >= NBUF:
                    # d slot free once the ACT square of chunk k-NBUF ran
                    vector.wait_ge(act_sem, k - NBUF + 1)
                vector.tensor_tensor(d_t[:, k % NBUF], a_t[:, k % NBUF],
                                     b_t[:, k % NBUF], SUB).then_inc(dve_sem, 1)
                if k == 2:
                    vector.wait_ge(ldma_sem, 16 * (S + 1))
                    vector.tensor_tensor(lp_t[:], lp_t[:], lb_t[:],
                                         SUB).then_inc(ldve_sem, 1)
            # end-game tiles (chunk 8 in slot 2, chunk 9 in slot 0)
            for k, c0, w in TILES:
                slot = k % NBUF
                if k == 8:
                    j = [0, 704].index(c0)
                    vector.wait_ge(c8_sems[j], 32)
                    if j == 0:
                        vector.wait_ge(act_sem, 8 - NBUF + 1)  # ACT(5)
                else:
                    j = [0, 704, 1184].index(c0)
                    vector.wait_ge(sub_sems[j], 32)
                    if j == 0:
                        vector.wait_ge(act_sem, 9 - NBUF + 1)  # ACT(6)
                vector.tensor_tensor(d_t[:, slot, c0:c0 + w],
                                     a_t[:, slot, c0:c0 + w],
                                     b_t[:, slot, c0:c0 + w],
                                     SUB).then_inc(dve_sem, 1)

        @block.scalar
        def _(act):
            # label I/O rides the ACT HWDGE queue, off the heat-load path
            act.dma_start(out=lp_t[:],
                          in_=lp.rearrange("s b c f -> (s b) (c f)")
                          ).then_inc(ldma_sem, 16)
            lb_src = lb.rearrange("b c f -> b (c f)")
            for s in range(S):
                act.dma_start(out=lb_t[s * BL:(s + 1) * BL, :],
                              in_=lb_src).then_inc(ldma_sem, 16)
            for k in range(NFULL):
                act.wait_ge(dve_sem, k + 1)
                seg = d_t[:, k % NBUF]
                # Square(d/H) summed over the free axis gives
                # sum(d^2)/(H*W) per partition — mean over (h,w) folded.
                act.activation(seg, seg, SQUARE, scale=1.0 / float(H),
                               accum_out=acc[:, k:k + 1]).then_inc(act_sem, 1)
                if k == 3:
                    act.wait_ge(ldve_sem, 1)
                    act.activation(lp_t[:], lp_t[:], SQUARE,
                                   accum_out=lacc[:]).then_inc(lact_sem, 1)
                    # gate the store on the accum write — the sequencer
                    # enqueues DMAs ahead of the ACT datapath
                    act.wait_ge(lact_sem, 1)
                    # partition p = s*BL + b matches [S, BL] row-major
                    act.dma_start(out=label_out.rearrange("s b -> (s b)"),
                                  in_=lacc[:]).then_inc(odma_sem, 16)
            # end-game tiles
            for i, (k, c0, w) in enumerate(TILES):
                act.wait_ge(dve_sem, NFULL + i + 1)
                seg = d_t[:, k % NBUF, c0:c0 + w]
                act.activation(seg, seg, SQUARE, scale=1.0 / float(H),
                               accum_out=acc[:, NFULL + i:NFULL + i + 1]
                               ).then_inc(act_sem, 1)
            # fold PSUM back to SBUF for the store: cols 0..11 right after
            # the first matmul (ACT is idle by then), the last col alone so
            # the post-stream chain only carries a 1-element copy
            act.wait_ge(mm_sem, 1)
            act.copy(red[0:1, 0:NCOLS - 1], pacc[0:1, 0:NCOLS - 1])
            act.wait_ge(mm_sem, 2)
            act.copy(red[0:1, NCOLS - 1:NCOLS],
                     pacc[0:1, NCOLS - 1:NCOLS]).then_inc(cp_sem, 1)

        @block.tensor
        def _(tensor):
            # partition reduction: ones[128,1].T @ acc -> [1, NCOLS] in PSUM.
            # Columns 0..NCOLS-2 reduce as soon as their ACTs are done
            # (under the stream); the last column right after its ACT.
            tensor.wait_ge(act_sem, NCOLS - 1)
            tensor.matmul(pacc[0:1, 0:NCOLS - 1], ones,
                          acc[:, 0:NCOLS - 1]).then_inc(mm_sem, 1)
            tensor.wait_ge(act_sem, NCOLS)
            tensor.matmul(pacc[0:1, NCOLS - 1:NCOLS], ones,
                          acc[:, NCOLS - 1:NCOLS]).then_inc(mm_sem, 1)

    return nc


_CACHE = {}


def _get_program():
    if "nc" not in _CACHE:
        _CACHE["nc"] = build_program()
    return _CACHE["nc"]


def make_in_maps(heat_preds, heatmaps, label_preds, labels):
    in_maps = []
    for i in range(NCORES):
        b0 = i * BL
        in_maps.append({
            "hp": np.ascontiguousarray(heat_preds[:, b0:b0 + BL], dtype=np.float32),
            "hm": np.ascontiguousarray(heatmaps[:, b0:b0 + BL], dtype=np.float32),
            "lp": np.ascontiguousarray(label_preds[:, b0:b0 + BL], dtype=np.float32),
            "lb": np.ascontiguousarray(labels[b0:b0 + BL], dtype=np.float32),
        })
    return in_maps


def run(heat_preds, heatmaps, label_preds, labels, trace=False, **spmd_kwargs):
    nc = _get_program()
    in_maps = make_in_maps(heat_preds, heatmaps, label_preds, labels)
    res = run_bass_kernel_spmd(nc, in_maps, list(range(NCORES)), trace=trace,
                               **spmd_kwargs)
    heat_rows = []
    for r in res.results:
        cols = r["heat_out"].reshape(-1)          # [NCOLS]
        per = np.concatenate([cols[:NFULL],
                              [cols[NFULL] + cols[NFULL + 1]],
                              [cols[NFULL + 2:].sum()]])  # [SB]
        heat_rows.append(per.reshape(S, BL).T)    # [BL, S]
    combined = np.concatenate(heat_rows, axis=0)
    labels_loss = np.concatenate([r["label_out"].T for r in res.results], axis=0)
    return (combined, labels_loss), res


def kernel(heat_preds, heatmaps, label_preds, labels):
    out, _ = run(heat_preds, heatmaps, label_preds, labels)
    return out


# revision 20
# speedup vs baseline: 1.1207x; 1.1207x over previous
"""Trainium2 Bass kernel for nn_KeypointLoss.

Reference computation (S=nstack+1=5, B=16, C=11, H=W=128, NCLASS=11, F=7):
  heat_l[b, s]   = sum_c mean_{h,w} (heat_preds[s,b,c,h,w] - heatmaps[s,b,c,h,w])^2
  labels_l[b, s] = sum_{c,f<7} (label_preds[s,b,c,f] - labels[b,c,f])^2
Returns (heat_l [B,S], labels_l [B,S]).

Sharding: data parallel over batch — core i handles b in {2i, 2i+1}.
Per core: read 2 x 7.2 MB of heatmap data (memory-bound). Raw Bass,
hand-scheduled. The DMA stream runs at the per-core HBM ceiling (~400
GB/s, all 16 SDMA engines saturated), so the design maximizes descriptor
size mid-stream and minimizes work after the last byte lands:
  * chunks 0..5 load as three 2-chunk pair transfers per tensor
    (11264 B descriptors — measured ~4% more per-engine DMA throughput
    than 5632 B, and fewer transfer boundaries). A pair lands chunk 2j
    in partitions 0..63 and chunk 2j+1 in partitions 64..127; its two
    1408-col column-tiles are exactly the per-slot compute tiles, so
    the TT/ACT pipeline is unchanged — only the final reduction must
    keep the partition halves separate.
  * chunks 6, 7 stay single (pairing them would backlog the end-game),
    chunk 8 is DMA-split [704, 704] and chunk 9 [704, 480, 224] so the
    post-stream chain is one small TT + ACT.
  * partition reduction on-core: PE matmul with a memset [128, 2]
    half-mask (rows 0..63 -> out row 0, 64..127 -> row 1), giving
    per-half sums that the host recombines per chunk. Columns 0..11
    reduce under the stream; the last column right after its ACT;
    split PSUM->SBUF copies; one ~100 B store from the idle SP queue.
  * label losses ride the ACT HWDGE queue early, hidden under the
    stream.
"""

import sys

if "/opt/trn_rl_repo" not in sys.path:
    sys.path.insert(0, "/opt/trn_rl_repo")

import numpy as np

import concourse.bass as bass
import concourse.mybir as mybir
from concourse.bass_utils import run_bass_kernel_spmd

S, B, C, H, W = 5, 16, 11, 128, 128
NCLASS, F = 11, 7
NCORES = 8
BL = B // NCORES          # batch samples per core (2)
P = 128                   # SBUF partitions
X = C * H * W // P        # 1408 free elements per partition per (s, b)
LF = NCLASS * F           # 77 label features per (s, b)
SB = S * BL               # 10 (s, b) pairs per core
NPAIR = 3                 # chunk pairs (0,1) (2,3) (4,5) as merged transfers
C8SUBS = [704, 704]       # chunk 8 DMA split (cols)
SUBS = [704, 480, 224]    # chunk 9 DMA split (cols)
NSLOT = 6                 # SBUF slots of [P, X]; pairs fill 2 adjacent slots
NCOLS = 2 * NPAIR + 2 + len(C8SUBS) + len(SUBS)   # 13 accumulator columns

f32 = mybir.dt.float32
SQUARE = mybir.ActivationFunctionType.Square
SUB = mybir.AluOpType.subtract


def build_program():
    nc = bass.Bass("TRN2")
    hp = nc.dram_tensor("hp", [S, BL, C, H, W], f32, kind="ExternalInput")
    hm = nc.dram_tensor("hm", [S, BL, C, H, W], f32, kind="ExternalInput")
    lp = nc.dram_tensor("lp", [S, BL, NCLASS, F], f32, kind="ExternalInput")
    lb = nc.dram_tensor("lb", [BL, NCLASS, F], f32, kind="ExternalInput")
    # heat_out[r, c] = partition-half r sum of accumulator column c; the
    # host recombines halves/columns into per-chunk losses
    heat_out = nc.dram_tensor("heat_out", [2, NCOLS], f32,
                              kind="ExternalOutput")
    label_out = nc.dram_tensor("label_out", [S, BL], f32, kind="ExternalOutput")

    # chunk k = (s, b) pair, 0.7 MB contiguous; chunk RANGES are also
    # contiguous, so a 2-chunk pair maps to [P, 2X] with 11264 B runs.
    def range_src(t, k0, nchunk, c0=0, w=None):
        cols = X * nchunk
        w = cols if w is None else w
        flat = t.rearrange("s b c h w -> (s b c h w)")
        n = cols * P
        ap = flat[k0 * X * P:k0 * X * P + n].rearrange("(p x) -> p x", p=P)
        return ap[:, c0:c0 + w]

    from contextlib import ExitStack
    with ExitStack() as es:
        a_t = es.enter_context(nc.sbuf_tensor([P, NSLOT, X], f32))
        b_t = es.enter_context(nc.sbuf_tensor([P, NSLOT, X], f32))
        d_t = es.enter_context(nc.sbuf_tensor([P, NSLOT, X], f32))
        acc = es.enter_context(nc.sbuf_tensor([P, NCOLS], f32))
        red = es.enter_context(nc.sbuf_tensor([2, NCOLS], f32))
        mask = es.enter_context(nc.sbuf_tensor([P, 2], f32))
        lp_t = es.enter_context(nc.sbuf_tensor([SB, LF], f32))
        lb_t = es.enter_context(nc.sbuf_tensor([SB, LF], f32))
        lacc = es.enter_context(nc.sbuf_tensor([SB, 1], f32))
        pacc = es.enter_context(nc.psum_tensor("pacc", [2, NCOLS], f32))
        # one sem per transfer pair: DMA completions are unordered across
        # transfers, so cumulative counting on one sem is racy
        pair_sems = [es.enter_context(nc.semaphore(f"pair_sem{j}"))
                     for j in range(NPAIR)]
        dma_sems = [es.enter_context(nc.semaphore(f"dma_sem{k}"))
                    for k in range(2)]                      # chunks 6, 7
        c8_sems = [es.enter_context(nc.semaphore(f"c8_sem{j}"))
                   for j in range(len(C8SUBS))]
        sub_sems = [es.enter_context(nc.semaphore(f"sub_sem{j}"))
                    for j in range(len(SUBS))]
        ldma_sem = es.enter_context(nc.semaphore("ldma_sem"))  # label loads
        odma_sem = es.enter_context(nc.semaphore("odma_sem"))  # output stores
        dve_sem = es.enter_context(nc.semaphore("dve_sem"))    # +1 per heat TT
        ldve_sem = es.enter_context(nc.semaphore("ldve_sem"))  # +1 label TT
        act_sem = es.enter_context(nc.semaphore("act_sem"))    # +1 per heat ACT
        lact_sem = es.enter_context(nc.semaphore("lact_sem"))  # +1 label ACT
        mm_sem = es.enter_context(nc.semaphore("mm_sem"))      # +1 per matmul
        cp_sem = es.enter_context(nc.semaphore("cp_sem"))      # PSUM->SBUF copy
        msk_sem = es.enter_context(nc.semaphore("msk_sem"))    # mask ready
        block = es.enter_context(nc.Block())

        @block.gpsimd
        def _(g):
            # [128, 2] half-mask for the partition reduction: rows 0..63
            # select into output row 0, rows 64..127 into row 1
            g.memset(mask[:, :], 0.0)
            g.memset(mask[0:P // 2, 0:1], 1.0)
            g.memset(mask[P // 2:P, 1:2], 1.0).then_inc(msk_sem, 1)

        @block.sync
        def _(sync):
            # heat loads only — keep this queue dense. Pairs first (no
            # recycle waits needed with 6 slots), then the end-game.
            for j in range(NPAIR):
                dst_a = a_t[:, 2 * j:2 * j + 2].rearrange("p s x -> p (s x)")
                dst_b = b_t[:, 2 * j:2 * j + 2].rearrange("p s x -> p (s x)")
                sync.dma_start(out=dst_a,
                               in_=range_src(hp, 2 * j, 2)
                               ).then_inc(pair_sems[j], 16)
                sync.dma_start(out=dst_b,
                               in_=range_src(hm, 2 * j, 2)
                               ).then_inc(pair_sems[j], 16)
            # chunk 6 -> slot 0 (freed by TT of pair-tile 0), 7 -> slot 1
            for i, k in enumerate((6, 7)):
                sync.wait_ge(dve_sem, i + 1)
                sync.dma_start(out=a_t[:, i],
                               in_=range_src(hp, k, 1)).then_inc(dma_sems[i], 16)
                sync.dma_start(out=b_t[:, i],
                               in_=range_src(hm, k, 1)).then_inc(dma_sems[i], 16)
            # chunk 8 sub-transfers -> slot 2 (freed by TT of pair-tile 2)
            sync.wait_ge(dve_sem, 3)
            c0 = 0
            for j, w in enumerate(C8SUBS):
                sync.dma_start(out=a_t[:, 2, c0:c0 + w],
                               in_=range_src(hp, 8, 1, c0, w)
                               ).then_inc(c8_sems[j], 16)
                sync.dma_start(out=b_t[:, 2, c0:c0 + w],
                               in_=range_src(hm, 8, 1, c0, w)
                               ).then_inc(c8_sems[j], 16)
                c0 += w
            # chunk 9 sub-transfers -> slot 3
            sync.wait_ge(dve_sem, 4)
            c0 = 0
            for j, w in enumerate(SUBS):
                sync.dma_start(out=a_t[:, 3, c0:c0 + w],
                               in_=range_src(hp, 9, 1, c0, w)
                               ).then_inc(sub_sems[j], 16)
                sync.dma_start(out=b_t[:, 3, c0:c0 + w],
                               in_=range_src(hm, 9, 1, c0, w)
                               ).then_inc(sub_sems[j], 16)
                c0 += w
            # final ~100 B store of the reduced losses (two descriptors)
            sync.wait_ge(cp_sem, 1)
            sync.dma_start(out=heat_out[:, :],
                           in_=red[0:2, :]).then_inc(odma_sem, 16)
            sync.wait_ge(odma_sem, 32)

        # TT/ACT tiles in order: (slot, col0, width, wait_fn)
        tiles = []
        for j in range(NPAIR):
            for h in (0, 1):
                tiles.append((2 * j + h, 0, X, ("pair", j)))
        tiles.append((0, 0, X, ("c67", 0)))
        tiles.append((1, 0, X, ("c67", 1)))
        c0 = 0
        for j, w in enumerate(C8SUBS):
            tiles.append((2, c0, w, ("c8", j)))
            c0 += w
        c0 = 0
        for j, w in enumerate(SUBS):
            tiles.append((3, c0, w, ("c9", j)))
            c0 += w

        @block.vector
        def _(vector):
            # heat TTs lead — the label path must never gate the heat
            # pipeline. The label TT slips in after the third tile, when
            # its loads are long done and the DVE has slack.
            for i, (slot, c0, w, gate) in enumerate(tiles):
                kind, j = gate
                if kind == "pair":
                    vector.wait_ge(pair_sems[j], 32)
                elif kind == "c67":
                    vector.wait_ge(dma_sems[j], 32)
                    # d slot recycle: freed by the ACT of pair-tile j
                    vector.wait_ge(act_sem, j + 1)
                elif kind == "c8":
                    vector.wait_ge(c8_sems[j], 32)
                    if j == 0:
                        vector.wait_ge(act_sem, 3)
                else:
                    vector.wait_ge(sub_sems[j], 32)
                    if j == 0:
                        vector.wait_ge(act_sem, 4)
                vector.tensor_tensor(d_t[:, slot, c0:c0 + w],
                                     a_t[:, slot, c0:c0 + w],
                                     b_t[:, slot, c0:c0 + w],
                                     SUB).then_inc(dve_sem, 1)
                if i == 2:
                    vector.wait_ge(ldma_sem, 16 * (S + 1))
                    vector.tensor_tensor(lp_t[:], lp_t[:], lb_t[:],
                                         SUB).then_inc(ldve_sem, 1)

        @block.scalar
        def _(act):
            # label I/O rides the ACT HWDGE queue, off the heat-load path
            act.dma_start(out=lp_t[:],
                          in_=lp.rearrange("s b c f -> (s b) (c f)")
                          ).then_inc(ldma_sem, 16)
            lb_src = lb.rearrange("b c f -> b (c f)")
            for s in range(S):
                act.dma_start(out=lb_t[s * BL:(s + 1) * BL, :],
                              in_=lb_src).then_inc(ldma_sem, 16)
            for i, (slot, c0, w, gate) in enumerate(tiles):
                act.wait_ge(dve_sem, i + 1)
                seg = d_t[:, slot, c0:c0 + w]
                # Square(d/H) summed over the free axis gives
                # sum(d^2)/(H*W) per partition — mean over (h,w) folded.
                act.activation(seg, seg, SQUARE, scale=1.0 / float(H),
                               accum_out=acc[:, i:i + 1]).then_inc(act_sem, 1)
                if i == 3:
                    act.wait_ge(ldve_sem, 1)
                    act.activation(lp_t[:], lp_t[:], SQUARE,
                                   accum_out=lacc[:]).then_inc(lact_sem, 1)
                    # gate the store on the accum write — the sequencer
                    # enqueues DMAs ahead of the ACT datapath
                    act.wait_ge(lact_sem, 1)
                    # partition p = s*BL + b matches [S, BL] row-major
                    act.dma_start(out=label_out.rearrange("s b -> (s b)"),
                                  in_=lacc[:]).then_inc(odma_sem, 16)
            # fold PSUM back to SBUF: cols 0..11 right after the first
            # matmul (ACT is idle by then), the last col alone
            act.wait_ge(mm_sem, 1)
            act.copy(red[0:2, 0:NCOLS - 1], pacc[0:2, 0:NCOLS - 1])
            act.wait_ge(mm_sem, 2)
            act.copy(red[0:2, NCOLS - 1:NCOLS],
                     pacc[0:2, NCOLS - 1:NCOLS]).then_inc(cp_sem, 1)

        @block.tensor
        def _(tensor):
            # partition-half reduction: mask[128,2].T @ acc -> [2, NCOLS]
            # in PSUM. Columns 0..NCOLS-2 reduce as soon as their ACTs are
            # done (under the stream); the last column right after its ACT.
            tensor.wait_ge(msk_sem, 1)
            tensor.wait_ge(act_sem, NCOLS - 1)
            tensor.matmul(pacc[0:2, 0:NCOLS - 1], mask[:, :],
                          acc[:, 0:NCOLS - 1]).then_inc(mm_sem, 1)
            tensor.wait_ge(act_sem, NCOLS)
            tensor.matmul(pacc[0:2, NCOLS - 1:NCOLS], mask[:, :],
                          acc[:, NCOLS - 1:NCOLS]).then_inc(mm_sem, 1)

    return nc


_CACHE = {}


def _get_program():
    if "nc" not in _CACHE:
        _CACHE["nc"] = build_program()
    return _CACHE["nc"]


def make_in_maps(heat_preds, heatmaps, label_preds, labels):
    in_maps = []
    for i in range(NCORES):
        b0 = i * BL
        in_maps.append({
            "hp": np.ascontiguousarray(heat_preds[:, b0:b0 + BL], dtype=np.float32),
            "hm": np.ascontiguousarray(heatmaps[:, b0:b0 + BL], dtype=np.float32),
            "lp": np.ascontiguousarray(label_preds[:, b0:b0 + BL], dtype=np.float32),
            "lb": np.ascontiguousarray(labels[b0:b0 + BL], dtype=np.float32),
        })
    return in_maps


def _fold_heat(red):
    """red [2, 13] half-sums -> per-chunk losses [SB]."""
    r0, r1 = red[0], red[1]
    per = np.empty(SB, dtype=red.dtype)
    for j in range(NPAIR):       # pair j col-tiles 2j, 2j+1
        per[2 * j] = r0[2 * j] + r0[2 * j + 1]
        per[2 * j + 1] = r1[2 * j] + r1[2 * j + 1]
    per[6] = r0[6] + r1[6]
    per[7] = r0[7] + r1[7]
    per[8] = r0[8] + r1[8] + r0[9] + r1[9]
    per[9] = red[:, 10:13].sum()
    return per


def run(heat_preds, heatmaps, label_preds, labels, trace=False, **spmd_kwargs):
    nc = _get_program()
    in_maps = make_in_maps(heat_preds, heatmaps, label_preds, labels)
    res = run_bass_kernel_spmd(nc, in_maps, list(range(NCORES)), trace=trace,
                               **spmd_kwargs)
    heat_rows = []
    for r in res.results:
        per = _fold_heat(r["heat_out"].reshape(2, NCOLS))
        heat_rows.append(per.reshape(S, BL).T)    # [BL, S]
    combined = np.concatenate(heat_rows, axis=0)
    labels_loss = np.concatenate([r["label_out"].T for r in res.results], axis=0)
    return (combined, labels_loss), res


def kernel(heat_preds, heatmaps, label_preds, labels):
    out, _ = run(heat_preds, heatmaps, label_preds, labels)
    return out
